# revision 43
# baseline (speedup 1.0000x reference)
"""Trainium2 Bass kernel for a BiQRNN3D layer.

reference math:
  gates = conv3d(x, W, SAME, 3x3x3) + b          x: [2,16,31,256,256] f32
  Z, F1, F2 = split(gates, 3, channel)           W: [48,16,3,3,3], b: [48]
  Z = tanh(Z); F1 = sigmoid(F1); F2 = sigmoid(F2)
  h_fwd: depth-forward  recurrence h = F1*h + (1-F1)*Z
  h_bwd: depth-backward recurrence h = F2*h + (1-F2)*Z
  out = h_fwd + h_bwd                            [2,16,31,256,256] f32

Distribution: H (=256) is sharded 32 rows per core across 8 NeuronCores
(SPMD, identical program; each core's x shard carries its 1-row conv halo
with global-edge zeros baked in by the host).

Per-core pipeline (v3):
  * conv as matmul, K=(kd,ci)=48 rows per h-copy. x tile partitions 0-47
    hold 3 kd-shifted copies at h rows h0+2t (block A), partitions 64-111
    at h0+1+2t (block B); partition 48 = ones (bias row), 49-63 zeros.
    Host stores x as [B, 34h, CIN, 33d, 258w] fp16 so each x-tile
    partition loads as ONE contiguous 16KB DMA run.
  * M=96: psum [2h x 48co, 2d x 256w]; 6 K=112 matmuls (p x kw) per tile.
  * psum evacuates (fp32->fp16 cast) into SBUF G[96, 31d, 256w].
  * TensorEngine transposes G[96, 128w-chunk] -> PSUM fp16 [128, 96],
    8 d-slices packed per psum bank; ACT (tanh/sigmoid, reading PSUM)
    writes j-merged scan tiles [128pix, 2j, 16co, 31d], F2 d-reversed.
  * DVE: g = (f-1)*z, tensor_tensor_scan (h = f*h - g) both directions;
    GpSimd: o = h1 + rev(h2) into a 4-chunk batch tile; one DMA per
    block writes out [128, S/128, HID, D] fp16 (3968B runs).
  * process-block work is interleaved into the next block's conv dc
    loop so psum evacs never queue behind a burst of scan work.
"""

from contextlib import ExitStack

import numpy as np

import concourse.bass as bass
import concourse.tile as tile
from concourse import bacc, mybir
from concourse.masks import make_identity

F32 = mybir.dt.float32
F16 = mybir.dt.float16
AF = mybir.ActivationFunctionType
ALU = mybir.AluOpType

N_CORES = 8
B = 2
CIN = 16
HID = 16
CO = 3 * HID            # 48
D = 31
H = 256
W = 256
HSH = H // N_CORES      # 32
HB = 2                  # output h rows per conv block
DC = 2                  # d slices per psum tile
DP = D + 2              # 33
WP = W + 2              # 258
S = B * HSH * W         # 16384
CHUNK = 128
NST = 6                 # stationary matrices
DG = 8                  # d slices per transpose psum group
NBLK = B * (HSH // HB)  # 32
NPSTEP = 18             # process steps per block (8 P + 8 half-scans + 2 out)


def _build_program():
    nc = bacc.Bacc("TRN2", target_bir_lowering=False, debug=False)

    x_dram = nc.dram_tensor("x", [B, HSH + 2, 3, CIN, D, WP], F16,
                            kind="ExternalInput").ap()
    wts = nc.dram_tensor("wts", [128, NST * 2 * CO], F16,
                         kind="ExternalInput").ap()
    aux = nc.dram_tensor("aux", [16, 2 * D * WP], F16,
                         kind="ExternalInput").ap()
    out = nc.dram_tensor("out", [CHUNK, S // CHUNK, HID, D], F16,
                         kind="ExternalOutput").ap()

    n_hblk = HSH // HB

    with tile.TileContext(nc) as tc, ExitStack() as ctx:
        wsb = nc.alloc_sbuf_tensor("wsb", [128, NST * 2 * CO], F16).ap()
        ident = nc.alloc_sbuf_tensor("ident", [128, 128], F16).ap()
        # x tile: [partition, t, d, w]; A rows (0-47) h=h0+2t, B rows
        # (64-111) h=h0+1+2t; row 48 ones (bias), 49-63 zeros.
        xbufs = [nc.alloc_sbuf_tensor(f"xb{i}", [112, 2, D, WP], F16).ap()
                 for i in range(2)]

        nc.sync.dma_start(wsb, wts)
        make_identity(nc, ident)
        for xb in xbufs:
            nc.sync.dma_start(
                xb[48:64].rearrange("p a b c -> p (a b c)"), aux)

        ps_pool = ctx.enter_context(tc.tile_pool(name="ps", bufs=4,
                                                 space="PSUM"))
        tp_pool = ctx.enter_context(tc.tile_pool(name="tp", bufs=4,
                                                 space="PSUM"))
        g_pool = ctx.enter_context(tc.tile_pool(name="gp", bufs=2))
        sc_pool = ctx.enter_context(tc.tile_pool(name="sc", bufs=4))

        n_dc = (D + DC - 1) // DC

        def load_x(k):
            b_i, hb_i = divmod(k, n_hblk)
            xb = xbufs[k % 2]
            h0 = hb_i * HB
            for kd in range(3):
                for t in range(2):
                    nc.sync.dma_start(
                        xb[kd * 16:kd * 16 + 16, t],
                        x_dram[b_i, h0 + 2 * t, kd])
                    nc.sync.dma_start(
                        xb[64 + kd * 16:64 + kd * 16 + 16, t],
                        x_dram[b_i, h0 + 1 + 2 * t, kd])

        def conv_dc(k, dc, G):
            xb = xbufs[k % 2]
            d0 = dc * DC
            dn = min(DC, D - d0)
            ps = ps_pool.tile([2 * CO, DC * W], F32, tag="ps")
            psv = ps[:, 0:dn * W].rearrange("p (d w) -> p d w", w=W)
            kk = 0
            for p in range(2):
                for kw in range(3):
                    nc.tensor.matmul(
                        psv,
                        wsb[0:112, kk * 96:(kk + 1) * 96],
                        xb[0:112, p, d0:d0 + dn, kw:kw + W],
                        start=(kk == 0), stop=(kk == NST - 1))
                    kk += 1
            if dc >= 9 and dc % 2 == 1:
                nc.vector.tensor_copy(G[:, d0:d0 + dn, :], psv)
            else:
                nc.scalar.copy(G[:, d0:d0 + dn, :], psv)

        def scan_chunk(o_super, q, j, zt2, fall):
            ztv = zt2[:, j]
            f1v = fall[:, j, 0:HID]
            g1 = sc_pool.tile([128, HID, D], F16, tag="g1")
            g2 = sc_pool.tile([128, HID, D], F16, tag="g2")
            h1 = sc_pool.tile([128, HID, D], F32, tag="h1")
            h2 = sc_pool.tile([128, HID, D], F32, tag="h2")
            nc.vector.scalar_tensor_tensor(
                g1[:], f1v, 1.0, ztv, ALU.subtract, ALU.mult)
            nc.vector.memset(fall[:, j, 0:HID, 0:1], 0.0)
            nc.vector.tensor_tensor_scan(
                h1[:].rearrange("p c d -> p (c d)"),
                f1v.rearrange("p c d -> p (c d)"),
                g1[:].rearrange("p c d -> p (c d)"),
                0.0, ALU.mult, ALU.subtract)
            yield
            # Backward scan runs over the WHOLE f2 block flat-reversed
            # (c and d both reversed); g2/h2 live in that order and the
            # final add un-reverses both dims.
            f2rr = fall[:, j, HID:2 * HID][:, ::-1, ::-1]
            nc.vector.scalar_tensor_tensor(
                g2[:], f2rr, 1.0, ztv[:, ::-1, ::-1], ALU.subtract, ALU.mult)
            nc.vector.memset(fall[:, j, HID:2 * HID, D - 1:D], 0.0)
            nc.vector.tensor_tensor_scan(
                h2[:].rearrange("p c d -> p (c d)"),
                fall[:, j, HID:2 * HID].rearrange("p c d -> p (c d)")[:, ::-1],
                g2[:].rearrange("p c d -> p (c d)"),
                0.0, ALU.mult, ALU.subtract)
            nc.gpsimd.tensor_add(o_super[:, q], h1[:], h2[:, ::-1, ::-1])

        act_tiles = {}

        def pgen_f(k, G):
            for wc in range(2):
                zt2 = sc_pool.tile([128, 2, HID, D], F16, tag="zt2",
                                   bufs=6, name=f"zt2_{k}_{wc}")
                fall = sc_pool.tile([128, 2, 2 * HID, D], F16, tag="fall",
                                    bufs=6, name=f"fall_{k}_{wc}")
                act_tiles[(k, wc)] = (zt2, fall)
                for dg in range(0, D, DG):
                    dn_g = min(DG, D - dg)
                    P = tp_pool.tile([128, DG * 96], F16, tag="P")
                    for i in range(dn_g):
                        nc.tensor.transpose(
                            P[:, i * 96:(i + 1) * 96],
                            G[0:96, dg + i, wc * 128:(wc + 1) * 128],
                            ident[0:96, 0:96])
                    Pv = P[:].rearrange("p (dd j c) -> p j c dd", j=2, c=48)
                    Pv = Pv[:, :, :, 0:dn_g]
                    nc.scalar.activation(
                        zt2[:, :, :, dg:dg + dn_g],
                        Pv[:, :, 0:HID, :], AF.Tanh)
                    nc.scalar.activation(
                        fall[:, :, :, dg:dg + dn_g],
                        Pv[:, :, HID:3 * HID, :], AF.Sigmoid)
                    yield

        def sgen_f(k):
            b_i, hb_i = divmod(k, n_hblk)
            h0 = hb_i * HB
            sb = (b_i * (HSH * W) + h0 * W) // CHUNK
            o_super = sc_pool.tile([128, 4, HID, D], F16, tag="os", bufs=2)
            for wc in range(2):
                zt2, fall = act_tiles.pop((k, wc))
                for j in range(2):
                    yield from scan_chunk(o_super, 2 * wc + j, j, zt2, fall)
                    yield
                nc.sync.dma_start(out[:, sb + 2 * wc:sb + 2 * wc + 2],
                                  o_super[:, 2 * wc:2 * wc + 2])
                yield

        def issue(gen):
            try:
                next(gen)
                return True
            except StopIteration:
                return False

        load_x(0)
        pgens, sgens = {}, {}
        for k in range(NBLK):
            if k + 1 < NBLK:
                load_x(k + 1)
            def rr(gens):
                live = [g for g in gens if g is not None]
                while live:
                    keep = []
                    for g in live:
                        try:
                            next(g)
                            yield
                            keep.append(g)
                        except StopIteration:
                            pass
                    live = keep

            steps = [rr([sgens.pop(k - 2, None), pgens.pop(k - 1, None)])]
            G = g_pool.tile([96, D, W], F16, tag="G", name=f"G{k}")
            issued = 0
            for dc in range(n_dc):
                conv_dc(k, dc, G)
                target = ((dc + 1) * NPSTEP) // n_dc
                while issued < target and steps:
                    if issue(steps[0]):
                        issued += 1
                    else:
                        steps.pop(0)
            while steps:
                if not issue(steps[0]):
                    steps.pop(0)
            pgens[k] = pgen_f(k, G)
            sgens[k] = sgen_f(k)
        for gen in [sgens.pop(NBLK - 2), pgens.pop(NBLK - 1),
                    sgens.pop(NBLK - 1)]:
            while issue(gen):
                pass

    nc.finalize()
    return nc


def _host_inputs(x, Wc, b):
    """x: [B, CIN, D, H, W] f32 full input. Returns list of 8 in_maps."""
    bf = np.float16
    # 6 stationaries: idx = p*3+kw, each [128, 96] with cols (j*48+co).
    # rows 0-47 (block A, x at tile-h 2p):   tap kh = 2p - j
    # rows 64-111 (block B, x at h+1):       tap kh = 2p + 1 - j
    wt = np.zeros((NST, 128, 2 * CO), np.float32)
    for p in range(2):
        for kw in range(3):
            idx = p * 3 + kw
            for j in range(2):
                c0 = j * CO
                for blk, khv in ((0, 2 * p - j), (64, 2 * p + 1 - j)):
                    if khv < 0 or khv > 2:
                        continue
                    for kd in range(3):
                        p0 = blk + kd * 16
                        wt[idx, p0:p0 + 16, c0:c0 + CO] = \
                            Wc[:, :, kd, khv, kw].T
    wt[0, 48, 0:CO] = b
    wt[0, 48, CO:2 * CO] = b
    wts = wt.transpose(1, 0, 2).reshape(128, NST * 2 * CO).astype(bf)
    auxa = np.zeros((16, 2 * D * WP), np.float32)
    auxa[0, :] = 1.0
    auxa = auxa.astype(bf)

    # x layout: [B, 34h', 3kd, CIN, 31d, 258w]; h'=0 is global row hs-1
    # (halo); copy kd holds x d-window [kd-1, kd+30); w index = x_w + 1;
    # edges zero. Each (kd, ci) partition load is one contiguous 16KB run
    # and a whole 16-partition call reads a contiguous 256KB block.
    xt = np.ascontiguousarray(x.transpose(0, 3, 1, 2, 4))  # [B,H,CIN,D,W]
    in_maps = []
    for c in range(N_CORES):
        hs, he = c * HSH, (c + 1) * HSH
        xp = np.zeros((B, HSH + 2, CIN, DP, WP), np.float32)
        lo = max(hs - 1, 0)
        hi = min(he + 1, H)
        xp[:, (lo - (hs - 1)):(hi - (hs - 1)), :, 1:D + 1, 1:W + 1] = \
            xt[:, lo:hi, :, :, :]
        x3 = np.stack([xp[:, :, :, kd:kd + D, :] for kd in range(3)],
                      axis=2).astype(bf)
        in_maps.append({"x": x3, "wts": wts, "aux": auxa})
    return in_maps


_PROGRAM = None


def _get_program():
    global _PROGRAM
    if _PROGRAM is None:
        _PROGRAM = _build_program()
    return _PROGRAM


def run_sharded(in_maps, trace=False, **kw):
    from concourse import bass_utils
    nc = _get_program()
    return bass_utils.run_bass_kernel_spmd(
        nc, in_maps, core_ids=list(range(N_CORES)), trace=trace, **kw)


def _assemble(results):
    outf = np.empty((B, HID, D, H, W), np.float32)
    for c in range(N_CORES):
        raw = np.asarray(results[c]["out"]).astype(np.float32)
        # [128, S/128, HID, D]; stored col order per block is (wc, j),
        # pixel chunk order is (j, wc): un-permute [0, 2, 1, 3].
        raw = raw.reshape(CHUNK, S // CHUNK // 4, 4, HID, D)
        raw = raw[:, :, [0, 2, 1, 3]].reshape(CHUNK, S // CHUNK, HID, D)
        o = raw.transpose(1, 0, 2, 3).reshape(B, HSH, W, HID, D)
        o = o.transpose(0, 3, 4, 1, 2)
        outf[:, :, :, c * HSH:(c + 1) * HSH, :] = o
    return outf


def kernel(x, W, b):
    x = np.asarray(x, np.float32)
    W = np.asarray(W, np.float32)
    b = np.asarray(b, np.float32)
    in_maps = _host_inputs(x, W, b)
    res = run_sharded(in_maps)
    return _assemble(res.results)


# revision 45
# speedup vs baseline: 1.2106x; 1.2106x over previous
"""Trainium2 Bass kernel for a BiQRNN3D layer.

reference math:
  gates = conv3d(x, W, SAME, 3x3x3) + b          x: [2,16,31,256,256] f32
  Z, F1, F2 = split(gates, 3, channel)           W: [48,16,3,3,3], b: [48]
  Z = tanh(Z); F1 = sigmoid(F1); F2 = sigmoid(F2)
  h_fwd: depth-forward  recurrence h = F1*h + (1-F1)*Z
  h_bwd: depth-backward recurrence h = F2*h + (1-F2)*Z
  out = h_fwd + h_bwd                            [2,16,31,256,256] f32

Distribution: H (=256) is sharded 32 rows per core across 8 NeuronCores
(SPMD, identical program; each core's x shard carries its 1-row conv halo
with global-edge zeros baked in by the host).

Per-core pipeline (v3):
  * conv as matmul, K=(kd,ci)=48 rows per h-copy. x tile partitions 0-47
    hold 3 kd-shifted copies at h rows h0+2t (block A), partitions 64-111
    at h0+1+2t (block B); partition 48 = ones (bias row), 49-63 zeros.
    Host stores x as [B, 34h, CIN, 33d, 258w] fp16 so each x-tile
    partition loads as ONE contiguous 16KB DMA run.
  * M=96: psum [2h x 48co, 2d x 256w]; 6 K=112 matmuls (p x kw) per tile.
  * psum evacuates (fp32->fp16 cast) into SBUF G[96, 31d, 256w].
  * TensorEngine transposes G[96, 128w-chunk] -> PSUM fp16 [128, 96],
    8 d-slices packed per psum bank; ACT (tanh/sigmoid, reading PSUM)
    writes j-merged scan tiles [128pix, 2j, 16co, 31d], F2 d-reversed.
  * DVE: g = (f-1)*z, tensor_tensor_scan (h = f*h - g) both directions;
    GpSimd: o = h1 + rev(h2) into a 4-chunk batch tile; one DMA per
    block writes out [128, S/128, HID, D] fp16 (3968B runs).
  * process-block work is interleaved into the next block's conv dc
    loop so psum evacs never queue behind a burst of scan work.
"""

from contextlib import ExitStack

import numpy as np

import concourse.bass as bass
import concourse.tile as tile
from concourse import bacc, mybir
from concourse.masks import make_identity

F32 = mybir.dt.float32
F16 = mybir.dt.float16
AF = mybir.ActivationFunctionType
ALU = mybir.AluOpType

N_CORES = 8
B = 2
CIN = 16
HID = 16
CO = 3 * HID            # 48
D = 31
H = 256
W = 256
HSH = H // N_CORES      # 32
HB = 2                  # output h rows per conv block
DC = 2                  # d slices per psum tile
DP = D + 2              # 33
WP = W + 2              # 258
S = B * HSH * W         # 16384
CHUNK = 128
NST = 6                 # stationary matrices
DG = 8                  # d slices per transpose psum group
NBLK = B * (HSH // HB)  # 32
NPSTEP = 18             # process steps per block (8 P + 8 half-scans + 2 out)


def _build_program():
    nc = bacc.Bacc("TRN2", target_bir_lowering=False, debug=False)

    x_dram = nc.dram_tensor("x", [B, HSH + 2, 3, CIN, D, WP], F16,
                            kind="ExternalInput").ap()
    wts = nc.dram_tensor("wts", [128, NST * 2 * CO], F16,
                         kind="ExternalInput").ap()
    aux = nc.dram_tensor("aux", [16, 2 * D * WP], F16,
                         kind="ExternalInput").ap()
    out = nc.dram_tensor("out", [CHUNK, S // CHUNK, HID, D], F16,
                         kind="ExternalOutput").ap()

    n_hblk = HSH // HB

    with tile.TileContext(nc) as tc, ExitStack() as ctx:
        wsb = nc.alloc_sbuf_tensor("wsb", [128, NST * 2 * CO], F16).ap()
        ident = nc.alloc_sbuf_tensor("ident", [128, 128], F16).ap()
        # x tile: [partition, t, d, w]; A rows (0-47) h=h0+2t, B rows
        # (64-111) h=h0+1+2t; row 48 ones (bias), 49-63 zeros.
        xbufs = [nc.alloc_sbuf_tensor(f"xb{i}", [112, 2, D, WP], F16).ap()
                 for i in range(2)]

        nc.sync.dma_start(wsb, wts)
        make_identity(nc, ident)
        for xb in xbufs:
            nc.sync.dma_start(
                xb[48:64].rearrange("p a b c -> p (a b c)"), aux)

        ps_pool = ctx.enter_context(tc.tile_pool(name="ps", bufs=4,
                                                 space="PSUM"))
        tp_pool = ctx.enter_context(tc.tile_pool(name="tp", bufs=4,
                                                 space="PSUM"))
        g_pool = ctx.enter_context(tc.tile_pool(name="gp", bufs=2))
        sc_pool = ctx.enter_context(tc.tile_pool(name="sc", bufs=4))

        n_dc = (D + DC - 1) // DC

        def load_x(k):
            b_i, hb_i = divmod(k, n_hblk)
            xb = xbufs[k % 2]
            h0 = hb_i * HB
            # block 0 loads in d-halves so conv(0) dc0-7 (d<16) can
            # start as soon as the first halves land
            dsplits = [(0, 16), (16, D)] if k == 0 else [(0, D)]
            for ds, de in dsplits:
                for kd in range(3):
                    for t in range(2):
                        nc.sync.dma_start(
                            xb[kd * 16:kd * 16 + 16, t, ds:de],
                            x_dram[b_i, h0 + 2 * t, kd, :, ds:de])
                        nc.sync.dma_start(
                            xb[64 + kd * 16:64 + kd * 16 + 16, t, ds:de],
                            x_dram[b_i, h0 + 1 + 2 * t, kd, :, ds:de])

        def conv_dc(k, dc, G):
            xb = xbufs[k % 2]
            d0 = dc * DC
            dn = min(DC, D - d0)
            ps = ps_pool.tile([2 * CO, DC * W], F32, tag="ps")
            psv = ps[:, 0:dn * W].rearrange("p (d w) -> p d w", w=W)
            kk = 0
            for p in range(2):
                for kw in range(3):
                    nc.tensor.matmul(
                        psv,
                        wsb[0:112, kk * 96:(kk + 1) * 96],
                        xb[0:112, p, d0:d0 + dn, kw:kw + W],
                        start=(kk == 0), stop=(kk == NST - 1))
                    kk += 1
            if dc >= 9 and dc % 2 == 1:
                nc.vector.tensor_copy(G[:, d0:d0 + dn, :], psv)
            else:
                nc.scalar.copy(G[:, d0:d0 + dn, :], psv)

        def scan_chunk(o_super, q, j, zt2, fall):
            ztv = zt2[:, j]
            f1v = fall[:, j, 0:HID]
            g1 = sc_pool.tile([128, HID, D], F16, tag="g1")
            g2 = sc_pool.tile([128, HID, D], F16, tag="g2")
            h1 = sc_pool.tile([128, HID, D], F32, tag="h1")
            h2 = sc_pool.tile([128, HID, D], F32, tag="h2")
            nc.vector.scalar_tensor_tensor(
                g1[:], f1v, 1.0, ztv, ALU.subtract, ALU.mult)
            nc.vector.memset(fall[:, j, 0:HID, 0:1], 0.0)
            nc.vector.tensor_tensor_scan(
                h1[:].rearrange("p c d -> p (c d)"),
                f1v.rearrange("p c d -> p (c d)"),
                g1[:].rearrange("p c d -> p (c d)"),
                0.0, ALU.mult, ALU.subtract)
            yield
            # Backward scan runs over the WHOLE f2 block flat-reversed
            # (c and d both reversed); g2/h2 live in that order and the
            # final add un-reverses both dims.
            f2rr = fall[:, j, HID:2 * HID][:, ::-1, ::-1]
            nc.vector.scalar_tensor_tensor(
                g2[:], f2rr, 1.0, ztv[:, ::-1, ::-1], ALU.subtract, ALU.mult)
            nc.vector.memset(fall[:, j, HID:2 * HID, D - 1:D], 0.0)
            nc.vector.tensor_tensor_scan(
                h2[:].rearrange("p c d -> p (c d)"),
                fall[:, j, HID:2 * HID].rearrange("p c d -> p (c d)")[:, ::-1],
                g2[:].rearrange("p c d -> p (c d)"),
                0.0, ALU.mult, ALU.subtract)
            nc.gpsimd.tensor_add(o_super[:, q], h1[:], h2[:, ::-1, ::-1])

        act_tiles = {}

        def pgen_f(k, G):
            for wc in range(2):
                zt2 = sc_pool.tile([128, 2, HID, D], F16, tag="zt2",
                                   bufs=6, name=f"zt2_{k}_{wc}")
                fall = sc_pool.tile([128, 2, 2 * HID, D], F16, tag="fall",
                                    bufs=6, name=f"fall_{k}_{wc}")
                act_tiles[(k, wc)] = (zt2, fall)
                for dg in range(0, D, DG):
                    dn_g = min(DG, D - dg)
                    P = tp_pool.tile([128, DG * 96], F16, tag="P")
                    for i in range(dn_g):
                        nc.tensor.transpose(
                            P[:, i * 96:(i + 1) * 96],
                            G[0:96, dg + i, wc * 128:(wc + 1) * 128],
                            ident[0:96, 0:96])
                    Pv = P[:].rearrange("p (dd j c) -> p j c dd", j=2, c=48)
                    Pv = Pv[:, :, :, 0:dn_g]
                    nc.scalar.activation(
                        zt2[:, :, :, dg:dg + dn_g],
                        Pv[:, :, 0:HID, :], AF.Tanh)
                    nc.scalar.activation(
                        fall[:, :, :, dg:dg + dn_g],
                        Pv[:, :, HID:3 * HID, :], AF.Sigmoid)
                    yield

        def sgen_f(k):
            b_i, hb_i = divmod(k, n_hblk)
            h0 = hb_i * HB
            sb = (b_i * (HSH * W) + h0 * W) // CHUNK
            o_super = sc_pool.tile([128, 4, HID, D], F16, tag="os", bufs=2)
            for wc in range(2):
                zt2, fall = act_tiles.pop((k, wc))
                for j in range(2):
                    yield from scan_chunk(o_super, 2 * wc + j, j, zt2, fall)
                    yield
                nc.sync.dma_start(out[:, sb + 2 * wc:sb + 2 * wc + 2],
                                  o_super[:, 2 * wc:2 * wc + 2])
                yield

        def issue(gen):
            try:
                next(gen)
                return True
            except StopIteration:
                return False

        load_x(0)
        pgens, sgens = {}, {}
        for k in range(NBLK):
            if k + 1 < NBLK:
                load_x(k + 1)
            steps = []
            if k - 2 in sgens:
                steps.append(sgens.pop(k - 2))
            if k - 1 in pgens:
                steps.append(pgens.pop(k - 1))
            G = g_pool.tile([96, D, W], F16, tag="G", name=f"G{k}")
            issued = 0
            for dc in range(n_dc):
                conv_dc(k, dc, G)
                target = ((dc + 1) * NPSTEP) // n_dc
                while issued < target and steps:
                    if issue(steps[0]):
                        issued += 1
                    else:
                        steps.pop(0)
            while steps:
                if not issue(steps[0]):
                    steps.pop(0)
            pgens[k] = pgen_f(k, G)
            sgens[k] = sgen_f(k)
        for gen in [sgens.pop(NBLK - 2), pgens.pop(NBLK - 1),
                    sgens.pop(NBLK - 1)]:
            while issue(gen):
                pass

    nc.finalize()
    return nc


def _host_inputs(x, Wc, b):
    """x: [B, CIN, D, H, W] f32 full input. Returns list of 8 in_maps."""
    bf = np.float16
    # 6 stationaries: idx = p*3+kw, each [128, 96] with cols (j*48+co).
    # rows 0-47 (block A, x at tile-h 2p):   tap kh = 2p - j
    # rows 64-111 (block B, x at h+1):       tap kh = 2p + 1 - j
    wt = np.zeros((NST, 128, 2 * CO), np.float32)
    for p in range(2):
        for kw in range(3):
            idx = p * 3 + kw
            for j in range(2):
                c0 = j * CO
                for blk, khv in ((0, 2 * p - j), (64, 2 * p + 1 - j)):
                    if khv < 0 or khv > 2:
                        continue
                    for kd in range(3):
                        p0 = blk + kd * 16
                        wt[idx, p0:p0 + 16, c0:c0 + CO] = \
                            Wc[:, :, kd, khv, kw].T
    wt[0, 48, 0:CO] = b
    wt[0, 48, CO:2 * CO] = b
    wts = wt.transpose(1, 0, 2).reshape(128, NST * 2 * CO).astype(bf)
    auxa = np.zeros((16, 2 * D * WP), np.float32)
    auxa[0, :] = 1.0
    auxa = auxa.astype(bf)

    # x layout: [B, 34h', 3kd, CIN, 31d, 258w]; h'=0 is global row hs-1
    # (halo); copy kd holds x d-window [kd-1, kd+30); w index = x_w + 1;
    # edges zero. Each (kd, ci) partition load is one contiguous 16KB run
    # and a whole 16-partition call reads a contiguous 256KB block.
    xt = np.ascontiguousarray(x.transpose(0, 3, 1, 2, 4))  # [B,H,CIN,D,W]
    in_maps = []
    for c in range(N_CORES):
        hs, he = c * HSH, (c + 1) * HSH
        xp = np.zeros((B, HSH + 2, CIN, DP, WP), np.float32)
        lo = max(hs - 1, 0)
        hi = min(he + 1, H)
        xp[:, (lo - (hs - 1)):(hi - (hs - 1)), :, 1:D + 1, 1:W + 1] = \
            xt[:, lo:hi, :, :, :]
        x3 = np.stack([xp[:, :, :, kd:kd + D, :] for kd in range(3)],
                      axis=2).astype(bf)
        in_maps.append({"x": x3, "wts": wts, "aux": auxa})
    return in_maps


_PROGRAM = None


def _get_program():
    global _PROGRAM
    if _PROGRAM is None:
        _PROGRAM = _build_program()
    return _PROGRAM


def run_sharded(in_maps, trace=False, **kw):
    from concourse import bass_utils
    nc = _get_program()
    return bass_utils.run_bass_kernel_spmd(
        nc, in_maps, core_ids=list(range(N_CORES)), trace=trace, **kw)


def _assemble(results):
    outf = np.empty((B, HID, D, H, W), np.float32)
    for c in range(N_CORES):
        raw = np.asarray(results[c]["out"]).astype(np.float32)
        # [128, S/128, HID, D]; stored col order per block is (wc, j),
        # pixel chunk order is (j, wc): un-permute [0, 2, 1, 3].
        raw = raw.reshape(CHUNK, S // CHUNK // 4, 4, HID, D)
        raw = raw[:, :, [0, 2, 1, 3]].reshape(CHUNK, S // CHUNK, HID, D)
        o = raw.transpose(1, 0, 2, 3).reshape(B, HSH, W, HID, D)
        o = o.transpose(0, 3, 4, 1, 2)
        outf[:, :, :, c * HSH:(c + 1) * HSH, :] = o
    return outf


def kernel(x, W, b):
    x = np.asarray(x, np.float32)
    W = np.asarray(W, np.float32)
    b = np.asarray(b, np.float32)
    in_maps = _host_inputs(x, W, b)
    res = run_sharded(in_maps)
    return _assemble(res.results)
